# revision 4
# baseline (speedup 1.0000x reference)
"""Trainium2 Bass kernel for grouped per-block linear:
    y[b, g] = sum_d x[b, g*6+d] * W[g, d] + b[g]
x: [4194304, 60] f32 -> y: [4194304, 10] f32

Strategy (pure data parallel, 8 cores):
  - shard x by batch into 8 contiguous row blocks of 524288 rows
  - per core: tiles of [128 partitions, T rows/partition], partition-major
    rows (partition p owns T consecutive rows) so every DMA is
    per-partition-contiguous in DRAM.
  - elementwise multiply x * broadcast(W) (split across DVE and GPSIMD
    engines), writing strided into a [T,10,7]-layout tmp whose 7th column
    holds the bias; one DVE tensor_reduce over the last axis of size 7
    produces y + bias directly.
  - memory-bound target: ~147 MB DMA traffic per core.
"""

import numpy as np

# ---------------- hardcoded problem constants ----------------
B_TOTAL = 4_194_304
N_CORES = 8
R = B_TOTAL // N_CORES  # 524288 rows per core
G = 10                  # groups
D = 6                   # group dim
DW = G * D              # 60 features per row
W7 = G * (D + 1)        # 70 = tmp row width (6 data + 1 bias col per group)
P = 128                 # partitions
T = 64                  # rows per partition per tile
TILE_ROWS = P * T       # 8192 rows per tile
N_TILES = R // TILE_ROWS  # 64 iterations
NTMP = 3                # rotating bias-initialized tmp buffers
# fraction of multiply instructions routed to GPSIMD (rest on DVE).
# DVE also does all reduces; GPSIMD TT is ~2x slower per element.
GPSIMD_MUL_EVERY = None  # set below via schedule list

# Iterations whose multiply runs on GPSIMD (Pool engine).  DVE does all
# reduces (~299us) + remaining muls; GPSIMD mul is ~2.08x DVE-cost.
# alpha ~= 0.7 balances both around ~375us, under the ~410us DMA roofline.
_N_GP = 45
_GP_SET = frozenset(
    int(round(i * N_TILES / _N_GP)) for i in range(_N_GP)
)

_CACHE = {}


def _build_bass():
    import concourse.bacc as bacc
    import concourse.mybir as mybir
    import concourse.tile as tile

    f32 = mybir.dt.float32
    nc = bacc.Bacc("TRN2", target_bir_lowering=False, debug=False)

    xs = nc.dram_tensor("xs", [R, DW], f32, kind="ExternalInput")
    wbc = nc.dram_tensor("wbc", [P, T * DW], f32, kind="ExternalInput")
    binit = nc.dram_tensor("binit", [P, T * W7], f32, kind="ExternalInput")
    ys = nc.dram_tensor("ys", [R, G], f32, kind="ExternalOutput")

    xs_r = xs[:, :].rearrange("(n p t) d -> n p (t d)", p=P, t=T)
    ys_r = ys[:, :].rearrange("(n p t) g -> n p (t g)", p=P, t=T)

    with tile.TileContext(nc) as tc:
        with (
            tc.tile_pool(name="consts", bufs=1) as cpool,
            tc.tile_pool(name="xin", bufs=3) as xpool,
            tc.tile_pool(name="tmps", bufs=1) as tpool,
            tc.tile_pool(name="yout", bufs=3) as ypool,
        ):
            wt = cpool.tile([P, T * DW], f32, tag="wbc")
            nc.sync.dma_start(wt, wbc[:, :])
            wt4 = wt.rearrange("p (t g d) -> p t g d", t=T, g=G, d=D)

            tmps = []
            for k in range(NTMP):
                tk = tpool.tile([P, T * W7], f32, tag=f"tmp{k}")
                nc.sync.dma_start(tk, binit[:, :])
                tmps.append(tk)

            for i in range(N_TILES):
                xt = xpool.tile([P, T * DW], f32, tag="x")
                nc.sync.dma_start(xt, xs_r[i])
                xt4 = xt.rearrange("p (t g d) -> p t g d", t=T, g=G, d=D)

                tmp = tmps[i % NTMP]
                tmp4 = tmp.rearrange("p (t g j) -> p t g j", t=T, g=G, j=D + 1)
                mul_out = tmp4[:, :, :, 0:D]
                eng = nc.gpsimd if i in _GP_SET else nc.vector
                eng.tensor_tensor(mul_out, xt4, wt4, mybir.AluOpType.mult)

                yt = ypool.tile([P, T * G], f32, tag="y")
                tmp3 = tmp.rearrange("p (tg j) -> p tg j", j=D + 1)
                nc.vector.tensor_reduce(
                    yt, tmp3, mybir.AxisListType.X, mybir.AluOpType.add
                )
                nc.scalar.dma_start(ys_r[i], yt)

    nc.compile()
    return nc


def _get_bass():
    if "nc" not in _CACHE:
        _CACHE["nc"] = _build_bass()
    return _CACHE["nc"]


def _host_consts(W, b):
    # wbc[p, t*60 + g*6 + d] = W[g, d]
    wflat = np.ascontiguousarray(W, dtype=np.float32).reshape(DW)
    wbc = np.tile(wflat, (P, T)).astype(np.float32)
    # binit[p, t*70 + g*7 + j] = b[g] if j == 6 else 0
    brow = np.zeros((G, D + 1), dtype=np.float32)
    brow[:, D] = np.asarray(b, dtype=np.float32)
    binit = np.tile(brow.reshape(W7), (P, T)).astype(np.float32)
    return np.ascontiguousarray(wbc), np.ascontiguousarray(binit)


def _run(x, W, b, **spmd_kwargs):
    from concourse import bass_utils

    x = np.ascontiguousarray(x, dtype=np.float32)
    assert x.shape == (B_TOTAL, DW), x.shape
    wbc, binit = _host_consts(W, b)

    nc = _get_bass()
    in_maps = []
    for c in range(N_CORES):
        shard = x[c * R : (c + 1) * R]
        in_maps.append({"xs": shard, "wbc": wbc, "binit": binit})

    res = bass_utils.run_bass_kernel_spmd(
        nc, in_maps, core_ids=list(range(N_CORES)), **spmd_kwargs
    )
    y = np.concatenate([r["ys"] for r in res.results], axis=0)
    return y, res


def kernel(x, W, b):
    return _run(x, W, b)[0]
